# revision 17
# baseline (speedup 1.0000x reference)
"""Trainium2 Bass kernel for nn_CrossAttention_24034636988611.

Cross-attention: q/k/v projections + per-head softmax(q k^T / sqrt(LH)) v +
output projection.  B=4, L=V=1024, LH=VH=1024, H=16 heads, head_dim=64.

Sharding (8 NeuronCores): batch x head-group.  Core c = (b, g) with b = c//2,
g = c%2 handles batch b and heads g*8..g*8+7 (a 512-wide slice of LH).  The
host gathers with out[b] = part[b,0] + part[b,1] (o_b added by g==0 only).

Strategy (cost model: matmul cycles = out_free_size x dtype_factor; bf16=1.0,
fp8e4+DoubleRow=0.5 with 2 K-subtiles per instr):
  - x is transposed and cast on the HOST: xlT8/xvT8 fp8 + xvTb bf16 arrive
    pre-transposed, removing all PE transposes and x psum-drain copies.
  - q/k projections and scores run fp8 DoubleRow (qw/kw pre-scaled x64 to
    dodge e4m3 subnormals; undone in the exp scale).  The projections emit a
    "DR layout" directly: psum partition p of tile (t', s) holds
    q^T[head 4t'+p//32, d = 32s + p%32], so the score matmul's lhsT/rhs
    [32, 2(sub), N] APs need no relayout (host permutes qw/kw columns).
  - v path, attention output, and output projection stay bf16.
  - Attention output is o[L,d]: lhsT = P^T chunk, rhs = v tile (N=64/instr);
    denominators via 1-column matmuls against ones into a pre-zeroed PSUM
    bank (start=False always: start_tensor_calc arms the whole 2KB region,
    clobbering sibling columns).  Normalize = per-partition broadcast mult
    on DVE; o transposed on PE (bf16 ident, 1 c/row) for the out projection.
  - exp on ACT over [128,1024] two-bank PSUM tiles; PE emission interleaves
    denoms (trailing scores by 2), v-proj chunks, and m_block pieces so the
    in-order PE queue never blocks the ACT exp stream.
"""

from collections import deque
from contextlib import ExitStack

import numpy as np

B = 4
LS = VS = 1024
VH = LH = 1024
H = 16
HD = 64
N_CORES = 8
GD = 512            # LH slice per core (8 heads)
WS = 64.0           # fp8 weight pre-scale (e4m3 subnormal avoidance)
SCALE_EXP = 1.0 / (32.0 * WS * WS)   # 1/sqrt(LH) / (WS_q * WS_k)

USE_F32R = True     # kept for test.py compat; ignored

_CACHE = {}


def _build(use_f32r: bool = True, dbg: bool = False):
    import concourse.tile as tile
    from concourse import bacc, mybir
    from concourse.masks import make_identity

    F32 = mybir.dt.float32
    BF16 = mybir.dt.bfloat16
    FP8 = mybir.dt.float8e4
    AF = mybir.ActivationFunctionType
    DR = mybir.MatmulPerfMode.DoubleRow
    ADD = mybir.AluOpType.add
    MULT = mybir.AluOpType.mult

    nc = bacc.Bacc("TRN2", target_bir_lowering=False, debug=False,
                   num_devices=N_CORES)

    xlT_d = nc.dram_tensor("xlT", [128, 8, 1024], FP8, kind="ExternalInput").ap()
    xvT_d = nc.dram_tensor("xvT", [128, 8, 1024], FP8, kind="ExternalInput").ap()
    xvTb_d = nc.dram_tensor("xvTb", [128, 8, 1024], BF16, kind="ExternalInput").ap()
    qw_d = nc.dram_tensor("qw", [128, 4, 2, 512], FP8, kind="ExternalInput").ap()
    kw_d = nc.dram_tensor("kw", [128, 4, 2, 512], FP8, kind="ExternalInput").ap()
    vw_d = nc.dram_tensor("vw", [128, 8, 512], BF16, kind="ExternalInput").ap()
    ow_d = nc.dram_tensor("ow", [128, 4, 1024], BF16, kind="ExternalInput").ap()
    qb_d = nc.dram_tensor("qb", [4, 128], F32, kind="ExternalInput").ap()
    kb_d = nc.dram_tensor("kb", [4, 128], F32, kind="ExternalInput").ap()
    vb_d = nc.dram_tensor("vb", [1, GD], F32, kind="ExternalInput").ap()
    ob_d = nc.dram_tensor("ob", [1, LH], F32, kind="ExternalInput").ap()
    out_d = nc.dram_tensor("out", [LS, LH], F32, kind="ExternalOutput").ap()
    if dbg:
        dbg_qdr = nc.dram_tensor("dbg_qdr", [128, 2, 2, 1024], FP8, kind="ExternalOutput").ap()
        dbg_kdr = nc.dram_tensor("dbg_kdr", [128, 2, 2, 1024], FP8, kind="ExternalOutput").ap()
        dbg_va = nc.dram_tensor("dbg_va", [128, 8, 8, HD], BF16, kind="ExternalOutput").ap()
        dbg_oc = nc.dram_tensor("dbg_oc", [128, 4, 1024], BF16, kind="ExternalOutput").ap()
        dbg_dn = nc.dram_tensor("dbg_dn", [128, 64], F32, kind="ExternalOutput").ap()

    with tile.TileContext(nc, trace_sim=False) as tc, ExitStack() as ctx:
        singles = ctx.enter_context(tc.tile_pool(name="singles", bufs=1))
        pt_pool = ctx.enter_context(tc.tile_pool(name="ptp", bufs=3))
        osb_pool = ctx.enter_context(tc.tile_pool(name="osb", bufs=2))
        out_pool = ctx.enter_context(tc.tile_pool(name="outp", bufs=3))
        ps_big = ctx.enter_context(tc.tile_pool(name="psbig", bufs=2, space="PSUM"))
        ps_o = ctx.enter_context(tc.tile_pool(name="pso", bufs=3, space="PSUM"))
        ps_d = ctx.enter_context(tc.tile_pool(name="psd", bufs=1, space="PSUM"))

        # ---- setup ----
        ident_bf = singles.tile([128, 128], BF16)
        make_identity(nc, ident_bf)
        ones_bf = singles.tile([128, 1], BF16)
        nc.vector.memset(ones_bf, 1.0)
        trash = singles.tile([128, 1], F32)
        # warm the ACT exp table before anything depends on it
        nc.scalar.activation(trash, ones_bf, AF.Exp, bias=0.0, scale=1.0)

        qb_sb = singles.tile([128, 4], F32)
        nc.gpsimd.dma_start(out=qb_sb, in_=qb_d.rearrange("t p -> p t"))
        kb_sb = singles.tile([128, 4], F32)
        nc.gpsimd.dma_start(out=kb_sb, in_=kb_d.rearrange("t p -> p t"))
        vb_sb = singles.tile([1, GD], F32)
        nc.gpsimd.dma_start(out=vb_sb, in_=vb_d)
        vb_bc = singles.tile([128, GD], F32)
        nc.gpsimd.partition_broadcast(vb_bc, vb_sb)
        ob_sb = singles.tile([1, LH], F32)
        nc.gpsimd.dma_start(out=ob_sb, in_=ob_d)
        ob_bc = singles.tile([128, LH], F32)
        nc.gpsimd.partition_broadcast(ob_bc, ob_sb)

        # ---- input DMAs, split across DGE queues and chunked for latency ----
        kw_sb = singles.tile([128, 4, 2, 512], FP8)
        nc.sync.dma_start(out=kw_sb, in_=kw_d)
        xvT8 = singles.tile([128, 8, 1024], FP8)
        nc.sync.dma_start(out=xvT8[:, 0:2, :], in_=xvT_d[:, 0:2, :])
        nc.sync.dma_start(out=xvT8[:, 2:4, :], in_=xvT_d[:, 2:4, :])
        nc.sync.dma_start(out=xvT8[:, 4:6, :], in_=xvT_d[:, 4:6, :])
        nc.sync.dma_start(out=xvT8[:, 6:8, :], in_=xvT_d[:, 6:8, :])
        qw_sb = singles.tile([128, 4, 2, 512], FP8)
        nc.scalar.dma_start(out=qw_sb, in_=qw_d)
        xlT8 = singles.tile([128, 8, 1024], FP8)
        nc.scalar.dma_start(out=xlT8[:, 0:2, :], in_=xlT_d[:, 0:2, :])
        nc.scalar.dma_start(out=xlT8[:, 2:4, :], in_=xlT_d[:, 2:4, :])
        nc.scalar.dma_start(out=xlT8[:, 4:6, :], in_=xlT_d[:, 4:6, :])
        nc.scalar.dma_start(out=xlT8[:, 6:8, :], in_=xlT_d[:, 6:8, :])
        xvTb = singles.tile([128, 8, 1024], BF16)
        nc.gpsimd.dma_start(out=xvTb, in_=xvTb_d)
        vw_sb = singles.tile([128, 8, 512], BF16)
        nc.gpsimd.dma_start(out=vw_sb, in_=vw_d)
        ow_sb = singles.tile([128, 4, 1024], BF16)
        nc.gpsimd.dma_start(out=ow_sb, in_=ow_d)

        qdr = singles.tile([128, 2, 2, 1024], FP8)  # [32j+dlow, t', s, L]
        kdr = singles.tile([128, 2, 2, 1024], FP8)  # [32j+dlow, t', s, V]
        v_aug = singles.tile([128, 8, 8, HD], BF16)  # [v%128, vt, h, d]
        o_cat = singles.tile([128, 4, 1024], BF16)   # [d%128, d//128, L]
        rc_sb = singles.tile([128, 64], F32)         # [L%128, l*32+m*8+h]

        # PE p-state warmup: keep the PE busy from t=0 so the projections run
        # at full clock (2.4 GHz needs ~3us of continuous PE busy).
        warm = ps_o.tile([128, 128], F32, tag="po", name="warm")
        for wi in range(14):
            nc.tensor.matmul(warm, lhsT=ident_bf, rhs=ident_bf,
                             start=True, stop=True, skip_group_check=True)

        # ---- q/k projections (fp8 DoubleRow) ----
        def proj_dr(w_sb, x8, dst, b_sb, tp, lbl):
            for s in range(2):
                for half in range(2):
                    psp = ps_o.tile([128, 512], F32, tag="po",
                                    name=f"pp_{lbl}_{tp}_{s}_{half}")
                    for kt2 in range(4):
                        nc.tensor.matmul(
                            psp,
                            lhsT=w_sb[:, kt2, :,
                                      (2 * tp + s) * 128:(2 * tp + s + 1) * 128],
                            rhs=x8[:, 2 * kt2:2 * kt2 + 2,
                                   half * 512:(half + 1) * 512],
                            perf_mode=DR,
                            start=(kt2 == 0), stop=(kt2 == 3),
                        )
                    nc.vector.tensor_scalar_add(
                        dst[:, tp, s, half * 512:(half + 1) * 512], psp,
                        b_sb[:, 2 * tp + s:2 * tp + s + 1])

        proj_dr(kw_sb, xvT8, kdr, kb_sb, 0, "k")
        proj_dr(qw_sb, xlT8, qdr, qb_sb, 0, "q")
        proj_dr(kw_sb, xvT8, kdr, kb_sb, 1, "k")
        proj_dr(qw_sb, xlT8, qdr, qb_sb, 1, "q")

        # ---- phase C machinery ----
        pt_tiles = {}
        dn = ps_d.tile([128, 64], F32, tag="pd")
        nc.vector.memset(dn, 0.0)

        def emit_score(l, hh, j, vtp):
            if (l, hh) not in pt_tiles:
                pt_tiles[(l, hh)] = pt_pool.tile(
                    [128, 4, 4, 1024], BF16, tag="pt", name=f"pt_{l}_{hh}")
            ptt = pt_tiles[(l, hh)]
            sps = ps_big.tile([128, 1024], F32, tag="big",
                              name=f"sps_{l}_{hh}_{j}_{vtp}")
            for vsel in range(2):
                vt = 2 * vtp + vsel
                nc.tensor.matmul(
                    sps[:, vsel * 512:(vsel + 1) * 512],
                    lhsT=kdr[32 * j:32 * j + 32, hh, :,
                             vt * 128:(vt + 1) * 128],
                    rhs=qdr[32 * j:32 * j + 32, hh, :,
                            l * 512:(l + 1) * 512],
                    perf_mode=DR, start=True, stop=True,
                    tile_position=(32 * j, 0),
                    skip_group_check=True,
                )
            nc.scalar.activation(ptt[:, j, vtp, :], sps, AF.Exp,
                                 bias=0.0, scale=SCALE_EXP)

        def emit_denoms(l, hh, j, vtp):
            ptt = pt_tiles[(l, hh)]
            h = 4 * hh + j
            for vsel in range(2):
                for m in range(4):
                    col = l * 32 + m * 8 + h
                    nc.tensor.matmul(
                        dn[:, col:col + 1],
                        lhsT=ptt[:, j, vtp,
                                 vsel * 512 + m * 128:vsel * 512 + (m + 1) * 128],
                        rhs=ones_bf,
                        start=False,
                        stop=(vtp == 3 and vsel == 1),
                        skip_group_check=True,
                    )

        def emit_vproj(vt):
            psv = ps_o.tile([128, 512], F32, tag="po", name=f"pv_{vt}")
            for kt in range(8):
                nc.tensor.matmul(
                    psv,
                    lhsT=xvTb[:, kt, vt * 128:(vt + 1) * 128],
                    rhs=vw_sb[:, kt, :],
                    start=(kt == 0), stop=(kt == 7),
                )
            nc.vector.tensor_tensor(
                out=v_aug[:, vt],
                in0=psv.rearrange("p (h d) -> p h d", h=8),
                in1=vb_bc.rearrange("p (h d) -> p h d", h=8),
                op=ADD)

        # m_block pieces: recip, attn-out halves per m, finisher per m
        mb_state = {}

        def mb_recip(l):
            nc.vector.reciprocal(rc_sb[:, l * 32:(l + 1) * 32],
                                 dn[:, l * 32:(l + 1) * 32])

        def mb_attn(l, m, hh):
            key = (l, m)
            if key not in mb_state:
                mb_state[key] = ps_o.tile([128, 512], F32, tag="po",
                                          name=f"ops_{l}_{m}")
            ops = mb_state[key]
            ptt = pt_tiles[(l, hh)]
            for j in range(4):
                h = 4 * hh + j
                for vtp in range(4):
                    for vsel in range(2):
                        nc.tensor.matmul(
                            ops[:, h * 64:(h + 1) * 64],
                            lhsT=ptt[:, j, vtp,
                                     vsel * 512 + m * 128:vsel * 512 + (m + 1) * 128],
                            rhs=v_aug[:, 2 * vtp + vsel, h, :],
                            start=(vtp == 0 and vsel == 0),
                            stop=(vtp == 3 and vsel == 1),
                            skip_group_check=True,
                        )

        def mb_finish(l, m):
            ops = mb_state[(l, m)]
            mo = 4 * l + m
            osb = osb_pool.tile([128, 512], BF16, tag="osb",
                                name=f"osb_{l}_{m}")
            rcb = rc_sb[:, l * 32 + m * 8:l * 32 + (m + 1) * 8]
            nc.vector.tensor_tensor(
                out=osb.rearrange("p (h d) -> p h d", h=8),
                in0=ops.rearrange("p (h d) -> p h d", h=8),
                in1=rcb[:, :, None].broadcast_to([128, 8, HD]),
                op=MULT)
            psT = ps_big.tile([128, 512], BF16, tag="big", name=f"psT_{l}_{m}")
            for cc in range(4):
                nc.tensor.matmul(
                    psT[:, cc * 128:(cc + 1) * 128],
                    lhsT=osb[:, cc * 128:(cc + 1) * 128],
                    rhs=ident_bf,
                    is_transpose=True, start=True, stop=True,
                    skip_group_check=True,
                )
            nc.vector.tensor_copy(
                out=o_cat[:, :, mo * 128:(mo + 1) * 128],
                in_=psT.rearrange("p (c x) -> p c x", c=4))
            for n in range(2):
                po = ps_o.tile([128, 512], F32, tag="po",
                               name=f"po_{l}_{m}_{n}")
                for cc in range(4):
                    nc.tensor.matmul(
                        po,
                        lhsT=o_cat[:, cc, mo * 128:(mo + 1) * 128],
                        rhs=ow_sb[:, cc, n * 512:(n + 1) * 512],
                        start=(cc == 0), stop=(cc == 3),
                    )
                ot = out_pool.tile([128, 512], F32, tag="outp",
                                   name=f"ot_{l}_{m}_{n}")
                nc.vector.tensor_tensor(
                    out=ot, in0=po, in1=ob_bc[:, n * 512:(n + 1) * 512],
                    op=ADD)
                nc.sync.dma_start(
                    out=out_d[mo * 128:(mo + 1) * 128,
                              n * 512:(n + 1) * 512],
                    in_=ot)

        def mb_pieces(l):
            yield lambda: mb_recip(l)
            for m in range(4):
                yield lambda m=m: mb_attn(l, m, 0)
                yield lambda m=m: mb_attn(l, m, 1)
                yield lambda m=m: mb_finish(l, m)

        # ---- interleaved emission ----
        sc_tiles = [(l, hh, j, vtp)
                    for l in range(2) for hh in range(2)
                    for j in range(4) for vtp in range(4)]
        fillers = deque()
        vproj_left = deque(range(8))
        for i, (l, hh, j, vtp) in enumerate(sc_tiles):
            if i == 32:
                fillers.extend(mb_pieces(0))
            emit_score(l, hh, j, vtp)
            if i >= 2:
                emit_denoms(*sc_tiles[i - 2])
            if i >= 2 and vproj_left and i % 2 == 0:
                emit_vproj(vproj_left.popleft())
            if i >= 34 and fillers:
                fillers.popleft()()
                if fillers and i % 2 == 1:
                    fillers.popleft()()
        emit_denoms(*sc_tiles[62])
        emit_denoms(*sc_tiles[63])
        while fillers:
            fillers.popleft()()
        for piece in mb_pieces(1):
            piece()

        if dbg:
            nc.sync.dma_start(out=dbg_qdr, in_=qdr)
            nc.sync.dma_start(out=dbg_kdr, in_=kdr)
            nc.sync.dma_start(out=dbg_va, in_=v_aug)
            nc.sync.dma_start(out=dbg_oc, in_=o_cat)
            dn_sb = singles.tile([128, 64], F32)
            nc.vector.tensor_copy(out=dn_sb, in_=dn)
            nc.sync.dma_start(out=dbg_dn, in_=dn_sb)

    nc.compile()
    return nc


def get_nc(use_f32r=USE_F32R):
    key = ("nc",)
    if key not in _CACHE:
        _CACHE[key] = _build(use_f32r)
    return _CACHE[key]


def make_in_maps(inputs, use_f32r=None):
    """Shard full inputs into 8 per-core input maps (core c = batch c//2,
    head-group c%2), with host-side transposes, dtype casts and weight
    layout permutes."""
    import ml_dtypes

    bf16 = ml_dtypes.bfloat16
    fp8 = ml_dtypes.float8_e4m3

    inp = {k: np.ascontiguousarray(np.asarray(v, dtype=np.float32))
           for k, v in inputs.items()}
    zeros_ob = np.zeros((1, LH), np.float32)

    def xT(x, dt):
        # [1024, 1024] -> [p, kt, L] = x^T tiled by VH-chunk
        return np.ascontiguousarray(
            x.T.reshape(8, 128, 1024).transpose(1, 0, 2).astype(dt))

    def qk_w(w):
        # [1024, 512] -> [pk, kt2, ksub, (t', s, j, dlow)] fp8, pre-scaled
        r = (w * WS).reshape(4, 2, 128, 2, 4, 2, 32)
        r = r.transpose(2, 0, 1, 3, 5, 4, 6).reshape(128, 4, 2, 512)
        return np.ascontiguousarray(r.astype(fp8))

    def qk_b(b):
        # [512] -> [4, 128]: row 2t'+s, col 32j+dlow
        r = (b * WS).reshape(2, 4, 2, 32).transpose(0, 2, 1, 3).reshape(4, 128)
        return np.ascontiguousarray(r)

    # x transposes shared across the two head-group cores of each batch
    xls = [xT(inp["l_hidden_states"][b], fp8) for b in range(B)]
    xvs8 = [xT(inp["v_hidden_states"][b], fp8) for b in range(B)]
    xvsb = [xT(inp["v_hidden_states"][b], bf16) for b in range(B)]

    in_maps = []
    for c in range(N_CORES):
        b, g = c // 2, c % 2
        gs = slice(g * GD, (g + 1) * GD)
        vw = inp["v_w"][:, gs].reshape(8, 128, GD).transpose(1, 0, 2)
        ow = inp["o_w"][gs, :].reshape(4, 128, LH).transpose(1, 0, 2)
        in_maps.append({
            "xlT": xls[b],
            "xvT": xvs8[b],
            "xvTb": xvsb[b],
            "qw": qk_w(inp["q_w"][:, gs]),
            "kw": qk_w(inp["k_w"][:, gs]),
            "vw": np.ascontiguousarray(vw.astype(bf16)),
            "ow": np.ascontiguousarray(ow.astype(bf16)),
            "qb": qk_b(inp["q_b"][gs]),
            "kb": qk_b(inp["k_b"][gs]),
            "vb": np.ascontiguousarray(inp["v_b"][gs].reshape(1, GD)),
            "ob": (np.ascontiguousarray(inp["o_b"].reshape(1, LH))
                   if g == 0 else zeros_ob),
        })
    return in_maps


def gather(results):
    """Sum the two head-group partials per batch."""
    out = np.empty((B, LS, LH), np.float32)
    for b in range(B):
        out[b] = results[2 * b]["out"] + results[2 * b + 1]["out"]
    return out


def kernel(**inputs) -> np.ndarray:
    from concourse.bass_utils import run_bass_kernel_spmd

    nc = get_nc()
    in_maps = make_in_maps(inputs)
    res = run_bass_kernel_spmd(nc, in_maps, core_ids=list(range(N_CORES)))
    return gather(res.results)


if __name__ == "__main__":
    rng = np.random.RandomState(0)
    s = 0.02
    inputs = {
        "v_hidden_states": rng.randn(B, VS, VH).astype(np.float32),
        "l_hidden_states": rng.randn(B, LS, LH).astype(np.float32),
        "q_w": (rng.randn(LH, LH) * s).astype(np.float32),
        "q_b": np.zeros(LH, np.float32),
        "k_w": (rng.randn(VH, LH) * s).astype(np.float32),
        "k_b": np.zeros(LH, np.float32),
        "v_w": (rng.randn(VH, LH) * s).astype(np.float32),
        "v_b": np.zeros(LH, np.float32),
        "o_w": (rng.randn(LH, LH) * s).astype(np.float32),
        "o_b": np.zeros(LH, np.float32),
    }
    out = kernel(**inputs)
    print("out", out.shape, out.dtype, float(np.abs(out).mean()))


# revision 18
# speedup vs baseline: 1.0782x; 1.0782x over previous
"""Trainium2 Bass kernel for nn_CrossAttention_24034636988611.

Cross-attention: q/k/v projections + per-head softmax(q k^T / sqrt(LH)) v +
output projection.  B=4, L=V=1024, LH=VH=1024, H=16 heads, head_dim=64.

Sharding (8 NeuronCores): batch x head-group.  Core c = (b, g) with b = c//2,
g = c%2 handles batch b and heads g*8..g*8+7 (a 512-wide slice of LH).  The
host gathers with out[b] = part[b,0] + part[b,1] (o_b added by g==0 only).

Strategy (cost model: matmul cycles = out_free_size x dtype_factor; bf16=1.0,
fp8e4+DoubleRow=0.5 with 2 K-subtiles per instr):
  - x is transposed and cast on the HOST: xlT8/xvT8 fp8 + xvTb bf16 arrive
    pre-transposed, removing all PE transposes and x psum-drain copies.
  - q/k projections and scores run fp8 DoubleRow (qw/kw pre-scaled x64 to
    dodge e4m3 subnormals; undone in the exp scale).  The projections emit a
    "DR layout" directly: psum partition p of tile (t', s) holds
    q^T[head 4t'+p//32, d = 32s + p%32], so the score matmul's lhsT/rhs
    [32, 2(sub), N] APs need no relayout (host permutes qw/kw columns).
  - v path, attention output, and output projection stay bf16.
  - Attention output is o[L,d]: lhsT = P^T chunk, rhs = v tile (N=64/instr);
    denominators via 1-column matmuls against ones into a pre-zeroed PSUM
    bank (start=False always: start_tensor_calc arms the whole 2KB region,
    clobbering sibling columns).  Normalize = per-partition broadcast mult
    on DVE; o transposed on PE (bf16 ident, 1 c/row) for the out projection.
  - exp on ACT over [128,1024] two-bank PSUM tiles; PE emission interleaves
    denoms (trailing scores by 2), v-proj chunks, and m_block pieces so the
    in-order PE queue never blocks the ACT exp stream.
"""

from collections import deque
from contextlib import ExitStack

import numpy as np

B = 4
LS = VS = 1024
VH = LH = 1024
H = 16
HD = 64
N_CORES = 8
GD = 512            # LH slice per core (8 heads)
WS = 64.0           # fp8 weight pre-scale (e4m3 subnormal avoidance)
SCALE_EXP = 1.0 / (32.0 * WS * WS)   # 1/sqrt(LH) / (WS_q * WS_k)

USE_F32R = True     # kept for test.py compat; ignored

_CACHE = {}


def _build(use_f32r: bool = True, dbg: bool = False):
    import concourse.tile as tile
    from concourse import bacc, mybir
    from concourse.masks import make_identity

    F32 = mybir.dt.float32
    BF16 = mybir.dt.bfloat16
    FP8 = mybir.dt.float8e4
    AF = mybir.ActivationFunctionType
    DR = mybir.MatmulPerfMode.DoubleRow
    ADD = mybir.AluOpType.add
    MULT = mybir.AluOpType.mult

    nc = bacc.Bacc("TRN2", target_bir_lowering=False, debug=False,
                   num_devices=N_CORES)

    xlT_d = nc.dram_tensor("xlT", [128, 8, 1024], FP8, kind="ExternalInput").ap()
    xvT_d = nc.dram_tensor("xvT", [128, 8, 1024], FP8, kind="ExternalInput").ap()
    xvTb_d = nc.dram_tensor("xvTb", [128, 8, 1024], BF16, kind="ExternalInput").ap()
    qw_d = nc.dram_tensor("qw", [128, 4, 2, 512], FP8, kind="ExternalInput").ap()
    kw_d = nc.dram_tensor("kw", [128, 4, 2, 512], FP8, kind="ExternalInput").ap()
    vw_d = nc.dram_tensor("vw", [128, 8, 512], BF16, kind="ExternalInput").ap()
    ow_d = nc.dram_tensor("ow", [128, 4, 1024], BF16, kind="ExternalInput").ap()
    qb_d = nc.dram_tensor("qb", [4, 128], F32, kind="ExternalInput").ap()
    kb_d = nc.dram_tensor("kb", [4, 128], F32, kind="ExternalInput").ap()
    vb_d = nc.dram_tensor("vb", [1, GD], F32, kind="ExternalInput").ap()
    ob_d = nc.dram_tensor("ob", [1, LH], F32, kind="ExternalInput").ap()
    out_d = nc.dram_tensor("out", [LS, LH], F32, kind="ExternalOutput").ap()
    if dbg:
        dbg_qdr = nc.dram_tensor("dbg_qdr", [128, 2, 2, 1024], FP8, kind="ExternalOutput").ap()
        dbg_kdr = nc.dram_tensor("dbg_kdr", [128, 2, 2, 1024], FP8, kind="ExternalOutput").ap()
        dbg_va = nc.dram_tensor("dbg_va", [128, 8, 8, HD], BF16, kind="ExternalOutput").ap()
        dbg_oc = nc.dram_tensor("dbg_oc", [128, 4, 1024], BF16, kind="ExternalOutput").ap()
        dbg_dn = nc.dram_tensor("dbg_dn", [128, 64], F32, kind="ExternalOutput").ap()

    with tile.TileContext(nc, trace_sim=False) as tc, ExitStack() as ctx:
        singles = ctx.enter_context(tc.tile_pool(name="singles", bufs=1))
        pt_pool = ctx.enter_context(tc.tile_pool(name="ptp", bufs=3))
        osb_pool = ctx.enter_context(tc.tile_pool(name="osb", bufs=2))
        out_pool = ctx.enter_context(tc.tile_pool(name="outp", bufs=3))
        ps_big = ctx.enter_context(tc.tile_pool(name="psbig", bufs=2, space="PSUM"))
        ps_o = ctx.enter_context(tc.tile_pool(name="pso", bufs=3, space="PSUM"))
        ps_d = ctx.enter_context(tc.tile_pool(name="psd", bufs=1, space="PSUM"))

        # ---- setup ----
        ident_bf = singles.tile([128, 128], BF16)
        make_identity(nc, ident_bf)
        ones_bf = singles.tile([128, 1], BF16)
        nc.vector.memset(ones_bf, 1.0)
        trash = singles.tile([128, 1], F32)
        # warm the ACT exp table before anything depends on it
        nc.scalar.activation(trash, ones_bf, AF.Exp, bias=0.0, scale=1.0)

        qb_sb = singles.tile([128, 4], F32)
        nc.gpsimd.dma_start(out=qb_sb, in_=qb_d.rearrange("t p -> p t"))
        kb_sb = singles.tile([128, 4], F32)
        nc.gpsimd.dma_start(out=kb_sb, in_=kb_d.rearrange("t p -> p t"))
        vb_sb = singles.tile([1, GD], F32)
        nc.gpsimd.dma_start(out=vb_sb, in_=vb_d)
        vb_bc = singles.tile([128, GD], F32)
        nc.gpsimd.partition_broadcast(vb_bc, vb_sb)
        ob_sb = singles.tile([1, LH], F32)
        nc.gpsimd.dma_start(out=ob_sb, in_=ob_d)
        ob_bc = singles.tile([128, LH], F32)
        nc.gpsimd.partition_broadcast(ob_bc, ob_sb)

        # ---- input DMAs, split across DGE queues and chunked for latency ----
        kw_sb = singles.tile([128, 4, 2, 512], FP8)
        nc.sync.dma_start(out=kw_sb, in_=kw_d)
        xvT8 = singles.tile([128, 8, 1024], FP8)
        nc.sync.dma_start(out=xvT8[:, 0:2, :], in_=xvT_d[:, 0:2, :])
        nc.sync.dma_start(out=xvT8[:, 2:4, :], in_=xvT_d[:, 2:4, :])
        nc.sync.dma_start(out=xvT8[:, 4:6, :], in_=xvT_d[:, 4:6, :])
        nc.sync.dma_start(out=xvT8[:, 6:8, :], in_=xvT_d[:, 6:8, :])
        qw_sb = singles.tile([128, 4, 2, 512], FP8)
        nc.scalar.dma_start(out=qw_sb, in_=qw_d)
        xlT8 = singles.tile([128, 8, 1024], FP8)
        nc.scalar.dma_start(out=xlT8[:, 0:2, :], in_=xlT_d[:, 0:2, :])
        nc.scalar.dma_start(out=xlT8[:, 2:4, :], in_=xlT_d[:, 2:4, :])
        nc.scalar.dma_start(out=xlT8[:, 4:6, :], in_=xlT_d[:, 4:6, :])
        nc.scalar.dma_start(out=xlT8[:, 6:8, :], in_=xlT_d[:, 6:8, :])
        xvTb = singles.tile([128, 8, 1024], BF16)
        nc.gpsimd.dma_start(out=xvTb, in_=xvTb_d)
        vw_sb = singles.tile([128, 8, 512], BF16)
        nc.gpsimd.dma_start(out=vw_sb, in_=vw_d)
        ow_sb = singles.tile([128, 4, 1024], BF16)
        nc.gpsimd.dma_start(out=ow_sb, in_=ow_d)

        qdr = singles.tile([128, 2, 2, 1024], FP8)  # [32j+dlow, t', s, L]
        kdr = singles.tile([128, 2, 2, 1024], FP8)  # [32j+dlow, t', s, V]
        v_aug = singles.tile([128, 8, 8, HD], BF16)  # [v%128, vt, h, d]
        o_cat = singles.tile([128, 4, 1024], BF16)   # [d%128, d//128, L]
        rc_sb = singles.tile([128, 64], F32)         # [L%128, l*32+m*8+h]

        # PE p-state warmup: keep the PE busy from t=0 so the projections run
        # at full clock (2.4 GHz needs ~3us of continuous PE busy).
        warm = ps_o.tile([128, 128], F32, tag="po", name="warm")
        for wi in range(14):
            nc.tensor.matmul(warm, lhsT=ident_bf, rhs=ident_bf,
                             start=True, stop=True, skip_group_check=True)

        # ---- q/k projections (fp8 DoubleRow) ----
        def proj_dr(w_sb, x8, dst, b_sb, tp, lbl):
            for s in range(2):
                for half in range(2):
                    psp = ps_o.tile([128, 512], F32, tag="po",
                                    name=f"pp_{lbl}_{tp}_{s}_{half}")
                    for kt2 in range(4):
                        nc.tensor.matmul(
                            psp,
                            lhsT=w_sb[:, kt2, :,
                                      (2 * tp + s) * 128:(2 * tp + s + 1) * 128],
                            rhs=x8[:, 2 * kt2:2 * kt2 + 2,
                                   half * 512:(half + 1) * 512],
                            perf_mode=DR,
                            start=(kt2 == 0), stop=(kt2 == 3),
                        )
                    nc.vector.tensor_scalar_add(
                        dst[:, tp, s, half * 512:(half + 1) * 512], psp,
                        b_sb[:, 2 * tp + s:2 * tp + s + 1])

        proj_dr(kw_sb, xvT8, kdr, kb_sb, 0, "k")
        proj_dr(qw_sb, xlT8, qdr, qb_sb, 0, "q")
        proj_dr(kw_sb, xvT8, kdr, kb_sb, 1, "k")
        proj_dr(qw_sb, xlT8, qdr, qb_sb, 1, "q")

        # ---- phase C machinery ----
        pt_tiles = {}
        dn = ps_d.tile([128, 64], F32, tag="pd")
        nc.vector.memset(dn, 0.0)

        def emit_score(l, hh, j, vtp):
            if (l, hh) not in pt_tiles:
                pt_tiles[(l, hh)] = pt_pool.tile(
                    [128, 4, 4, 1024], BF16, tag="pt", name=f"pt_{l}_{hh}")
            ptt = pt_tiles[(l, hh)]
            sps = ps_big.tile([128, 1024], F32, tag="big",
                              name=f"sps_{l}_{hh}_{j}_{vtp}")
            for vsel in range(2):
                vt = 2 * vtp + vsel
                nc.tensor.matmul(
                    sps[:, vsel * 512:(vsel + 1) * 512],
                    lhsT=kdr[32 * j:32 * j + 32, hh, :,
                             vt * 128:(vt + 1) * 128],
                    rhs=qdr[32 * j:32 * j + 32, hh, :,
                            l * 512:(l + 1) * 512],
                    perf_mode=DR, start=True, stop=True,
                    tile_position=(32 * j, 0),
                    skip_group_check=True,
                )
            nc.scalar.activation(ptt[:, j, vtp, :], sps, AF.Exp,
                                 bias=0.0, scale=SCALE_EXP)

        def emit_denoms(l, hh, j, vtp):
            ptt = pt_tiles[(l, hh)]
            h = 4 * hh + j
            for vsel in range(2):
                for m in range(4):
                    col = l * 32 + m * 8 + h
                    nc.tensor.matmul(
                        dn[:, col:col + 1],
                        lhsT=ptt[:, j, vtp,
                                 vsel * 512 + m * 128:vsel * 512 + (m + 1) * 128],
                        rhs=ones_bf,
                        start=False,
                        stop=(vtp == 3 and vsel == 1),
                        skip_group_check=True,
                    )

        def emit_vproj(vt):
            psv = ps_o.tile([128, 512], F32, tag="po", name=f"pv_{vt}")
            for kt in range(8):
                nc.tensor.matmul(
                    psv,
                    lhsT=xvTb[:, kt, vt * 128:(vt + 1) * 128],
                    rhs=vw_sb[:, kt, :],
                    start=(kt == 0), stop=(kt == 7),
                )
            nc.vector.tensor_tensor(
                out=v_aug[:, vt],
                in0=psv.rearrange("p (h d) -> p h d", h=8),
                in1=vb_bc.rearrange("p (h d) -> p h d", h=8),
                op=ADD)

        # m_block pieces: recip, attn-out halves per m, finisher per m
        mb_state = {}

        def mb_recip(l):
            nc.vector.reciprocal(rc_sb[:, l * 32:(l + 1) * 32],
                                 dn[:, l * 32:(l + 1) * 32])

        def mb_attn(l, m, hh):
            key = (l, m)
            if key not in mb_state:
                mb_state[key] = ps_o.tile([128, 512], F32, tag="po",
                                          name=f"ops_{l}_{m}")
            ops = mb_state[key]
            ptt = pt_tiles[(l, hh)]
            for j in range(4):
                h = 4 * hh + j
                for vtp in range(4):
                    for vsel in range(2):
                        nc.tensor.matmul(
                            ops[:, h * 64:(h + 1) * 64],
                            lhsT=ptt[:, j, vtp,
                                     vsel * 512 + m * 128:vsel * 512 + (m + 1) * 128],
                            rhs=v_aug[:, 2 * vtp + vsel, h, :],
                            start=(vtp == 0 and vsel == 0),
                            stop=(vtp == 3 and vsel == 1),
                            skip_group_check=True,
                        )

        def mb_finish(l, m):
            ops = mb_state[(l, m)]
            mo = 4 * l + m
            osb = osb_pool.tile([128, 512], BF16, tag="osb",
                                name=f"osb_{l}_{m}")
            rcb = rc_sb[:, l * 32 + m * 8:l * 32 + (m + 1) * 8]
            nc.vector.tensor_tensor(
                out=osb.rearrange("p (h d) -> p h d", h=8),
                in0=ops.rearrange("p (h d) -> p h d", h=8),
                in1=rcb[:, :, None].broadcast_to([128, 8, HD]),
                op=MULT)
            psT = ps_big.tile([128, 512], BF16, tag="big", name=f"psT_{l}_{m}")
            for cc in range(4):
                nc.tensor.matmul(
                    psT[:, cc * 128:(cc + 1) * 128],
                    lhsT=osb[:, cc * 128:(cc + 1) * 128],
                    rhs=ident_bf,
                    is_transpose=True, start=True, stop=True,
                    skip_group_check=True,
                )
            nc.vector.tensor_copy(
                out=o_cat[:, :, mo * 128:(mo + 1) * 128],
                in_=psT.rearrange("p (c x) -> p c x", c=4))
            for n in range(2):
                po = ps_big.tile([128, 512], F32, tag="big",
                               name=f"po_{l}_{m}_{n}")
                for cc in range(4):
                    nc.tensor.matmul(
                        po,
                        lhsT=o_cat[:, cc, mo * 128:(mo + 1) * 128],
                        rhs=ow_sb[:, cc, n * 512:(n + 1) * 512],
                        start=(cc == 0), stop=(cc == 3),
                    )
                ot = out_pool.tile([128, 512], F32, tag="outp",
                                   name=f"ot_{l}_{m}_{n}")
                nc.vector.tensor_tensor(
                    out=ot, in0=po, in1=ob_bc[:, n * 512:(n + 1) * 512],
                    op=ADD)
                nc.sync.dma_start(
                    out=out_d[mo * 128:(mo + 1) * 128,
                              n * 512:(n + 1) * 512],
                    in_=ot)

        def mb_pieces(l):
            yield lambda: mb_recip(l)
            for m in range(4):
                yield lambda m=m: mb_attn(l, m, 0)
                yield lambda m=m: mb_attn(l, m, 1)
                yield lambda m=m: mb_finish(l, m)

        # ---- interleaved emission ----
        sc_tiles = [(l, hh, j, vtp)
                    for l in range(2) for hh in range(2)
                    for j in range(4) for vtp in range(4)]
        fillers = deque()
        vproj_left = deque(range(8))
        for i, (l, hh, j, vtp) in enumerate(sc_tiles):
            if i == 32:
                fillers.extend(mb_pieces(0))
            emit_score(l, hh, j, vtp)
            if i >= 2:
                emit_denoms(*sc_tiles[i - 2])
            if i >= 2 and vproj_left and i % 2 == 0:
                emit_vproj(vproj_left.popleft())
            if i >= 34 and fillers:
                fillers.popleft()()
                if fillers and i % 2 == 1:
                    fillers.popleft()()
        emit_denoms(*sc_tiles[62])
        emit_denoms(*sc_tiles[63])
        while fillers:
            fillers.popleft()()
        for piece in mb_pieces(1):
            piece()

        if dbg:
            nc.sync.dma_start(out=dbg_qdr, in_=qdr)
            nc.sync.dma_start(out=dbg_kdr, in_=kdr)
            nc.sync.dma_start(out=dbg_va, in_=v_aug)
            nc.sync.dma_start(out=dbg_oc, in_=o_cat)
            dn_sb = singles.tile([128, 64], F32)
            nc.vector.tensor_copy(out=dn_sb, in_=dn)
            nc.sync.dma_start(out=dbg_dn, in_=dn_sb)

    nc.compile()
    return nc


def get_nc(use_f32r=USE_F32R):
    key = ("nc",)
    if key not in _CACHE:
        _CACHE[key] = _build(use_f32r)
    return _CACHE[key]


def make_in_maps(inputs, use_f32r=None):
    """Shard full inputs into 8 per-core input maps (core c = batch c//2,
    head-group c%2), with host-side transposes, dtype casts and weight
    layout permutes."""
    import ml_dtypes

    bf16 = ml_dtypes.bfloat16
    fp8 = ml_dtypes.float8_e4m3

    inp = {k: np.ascontiguousarray(np.asarray(v, dtype=np.float32))
           for k, v in inputs.items()}
    zeros_ob = np.zeros((1, LH), np.float32)

    def xT(x, dt):
        # [1024, 1024] -> [p, kt, L] = x^T tiled by VH-chunk
        return np.ascontiguousarray(
            x.T.reshape(8, 128, 1024).transpose(1, 0, 2).astype(dt))

    def qk_w(w):
        # [1024, 512] -> [pk, kt2, ksub, (t', s, j, dlow)] fp8, pre-scaled
        r = (w * WS).reshape(4, 2, 128, 2, 4, 2, 32)
        r = r.transpose(2, 0, 1, 3, 5, 4, 6).reshape(128, 4, 2, 512)
        return np.ascontiguousarray(r.astype(fp8))

    def qk_b(b):
        # [512] -> [4, 128]: row 2t'+s, col 32j+dlow
        r = (b * WS).reshape(2, 4, 2, 32).transpose(0, 2, 1, 3).reshape(4, 128)
        return np.ascontiguousarray(r)

    # x transposes shared across the two head-group cores of each batch
    xls = [xT(inp["l_hidden_states"][b], fp8) for b in range(B)]
    xvs8 = [xT(inp["v_hidden_states"][b], fp8) for b in range(B)]
    xvsb = [xT(inp["v_hidden_states"][b], bf16) for b in range(B)]

    in_maps = []
    for c in range(N_CORES):
        b, g = c // 2, c % 2
        gs = slice(g * GD, (g + 1) * GD)
        vw = inp["v_w"][:, gs].reshape(8, 128, GD).transpose(1, 0, 2)
        ow = inp["o_w"][gs, :].reshape(4, 128, LH).transpose(1, 0, 2)
        in_maps.append({
            "xlT": xls[b],
            "xvT": xvs8[b],
            "xvTb": xvsb[b],
            "qw": qk_w(inp["q_w"][:, gs]),
            "kw": qk_w(inp["k_w"][:, gs]),
            "vw": np.ascontiguousarray(vw.astype(bf16)),
            "ow": np.ascontiguousarray(ow.astype(bf16)),
            "qb": qk_b(inp["q_b"][gs]),
            "kb": qk_b(inp["k_b"][gs]),
            "vb": np.ascontiguousarray(inp["v_b"][gs].reshape(1, GD)),
            "ob": (np.ascontiguousarray(inp["o_b"].reshape(1, LH))
                   if g == 0 else zeros_ob),
        })
    return in_maps


def gather(results):
    """Sum the two head-group partials per batch."""
    out = np.empty((B, LS, LH), np.float32)
    for b in range(B):
        out[b] = results[2 * b]["out"] + results[2 * b + 1]["out"]
    return out


def kernel(**inputs) -> np.ndarray:
    from concourse.bass_utils import run_bass_kernel_spmd

    nc = get_nc()
    in_maps = make_in_maps(inputs)
    res = run_bass_kernel_spmd(nc, in_maps, core_ids=list(range(N_CORES)))
    return gather(res.results)


if __name__ == "__main__":
    rng = np.random.RandomState(0)
    s = 0.02
    inputs = {
        "v_hidden_states": rng.randn(B, VS, VH).astype(np.float32),
        "l_hidden_states": rng.randn(B, LS, LH).astype(np.float32),
        "q_w": (rng.randn(LH, LH) * s).astype(np.float32),
        "q_b": np.zeros(LH, np.float32),
        "k_w": (rng.randn(VH, LH) * s).astype(np.float32),
        "k_b": np.zeros(LH, np.float32),
        "v_w": (rng.randn(VH, LH) * s).astype(np.float32),
        "v_b": np.zeros(LH, np.float32),
        "o_w": (rng.randn(LH, LH) * s).astype(np.float32),
        "o_b": np.zeros(LH, np.float32),
    }
    out = kernel(**inputs)
    print("out", out.shape, out.dtype, float(np.abs(out).mean()))
